# revision 1
# baseline (speedup 1.0000x reference)
"""MoE layer (top-2 of 8 experts, selection shared across tokens) on 8 TRN2 cores.

Math (faithful to the reference):
    gates = softmax(x @ W_gate + b_gate)          [N, 8]
    idx0  = top-2 expert indices of token 0       [2]
    s     = per-token top-2 gate VALUES (desc)    [N, 2]
    out   = s0 * (x @ W[A] + b[A]) + s1 * (x @ W[B] + b[B])

Strategy: gating + top-2 is 0.2% of the FLOPs -> computed on host.  The two
active expert matmuls (275 GFLOP) are data-parallel sharded over tokens across
8 cores; expert weights are replicated.  Matmuls run in fp16 (values are small,
so fp16 range is safe and its 10-bit mantissa keeps rel-err ~3e-4),
accumulating fp32 in PSUM.
"""

import contextlib
import ctypes
import functools
import os
import sys
import types

import numpy as np

import concourse.bass as bass
import concourse.mybir as mybir
import concourse.tile as tile
from concourse import bacc
from concourse import bass_utils as _bass_utils
from concourse.bass_utils import run_bass_kernel_spmd

# The A/B expert matmuls share the same stationary x-chunk, but walrus emits a
# (redundant) LDWEIGHTS before every MATMUL, and for 4-byte dtypes the LDW
# path (187ns + ~40ns handoff) gates the matmul cadence.  Walrus has a dedup
# pass for exactly this, hardcoded off in bir_verify_and_optimise.  Only
# useful for f32r kernels: with 16-bit operands the (FWL-style) LDWEIGHTS is
# rejected by that pass and is fully hidden anyway.
ENABLE_LDW_OPT = False
if ENABLE_LDW_OPT and not getattr(_bass_utils, "_ldw_opt_patch", False):
    _orig_run_command = _bass_utils.run_command

    def _run_command_ldw_opt(argv, **kwargs):
        argv = [
            "--enable-ldw-opt=true" if a == "--enable-ldw-opt=false" else a
            for a in argv
        ]
        return _orig_run_command(argv, **kwargs)

    _bass_utils.run_command = _run_command_ldw_opt
    _bass_utils._ldw_opt_patch = True

N_CORES = 8
N, D_IN, D_HID = 16384, 2048, 2048
NT = N // N_CORES            # tokens per core
KP = 128                     # contraction chunk = partition dim
KCH = D_IN // KP             # 16 K-chunks
NB = 512                     # output column block (1 PSUM bank of fp32)
NBLK = D_HID // NB           # 4 output blocks
TQ = 256                     # token slice per x-stream piece
NQ = NT // TQ                # 8 slices
MPQ = TQ // 128              # m-tiles per slice

F32 = mybir.dt.float32
F32R = mybir.dt.float32r
BF16 = mybir.dt.bfloat16

FP16 = mybir.dt.float16

# The PE streams one moving-operand column per cycle regardless of dtype, but
# 16-bit operands keep the (FWL) weight load fully hidden (97ns vs 187ns) and
# halve DMA.  fp16 (10 mantissa bits) beats bf16 ~8x on accuracy at identical
# speed, and the value ranges here (|x|<~6, |W|<~0.12) are safely inside
# fp16 range.  PSUM accumulates fp32; the per-token top-2 scores applied in
# the epilogue stay fp32.
W_DT = FP16
X_DT = FP16

# Filled by test harness inspection: last BassKernelResults from a run.
LAST_RESULT = None


@functools.lru_cache(maxsize=1)
def _build():
    nc = bacc.Bacc("TRN2", target_bir_lowering=False, debug=False)
    xT = nc.dram_tensor("xT", [D_IN, NT], X_DT, kind="ExternalInput")
    wa = nc.dram_tensor("wa", [D_IN, D_HID], W_DT, kind="ExternalInput")
    wb = nc.dram_tensor("wb", [D_IN, D_HID], W_DT, kind="ExternalInput")
    # bias pre-replicated across partitions on host: brep[p, e, o] = b_sel[e, o]
    brep = nc.dram_tensor("brep", [128, 2, D_HID], F32, kind="ExternalInput")
    # per-token scores pre-arranged on host, partition-major:
    # sC[p, m, s] = top2_score[m*128 + p, s]
    sC = nc.dram_tensor("sC", [128, NT // 128, 2], F32, kind="ExternalInput")
    out = nc.dram_tensor("out", [NT, D_HID], F32, kind="ExternalOutput")

    MULT = mybir.AluOpType.mult
    ADD = mybir.AluOpType.add

    with tile.TileContext(nc) as tc:
        with (
            tc.tile_pool(name="cst", bufs=1) as cst,
            tc.tile_pool(name="wp", bufs=2) as wp,
            tc.tile_pool(name="xp", bufs=3) as xp,
            tc.tile_pool(name="ep", bufs=2) as ep,
            tc.tile_pool(name="ps", bufs=3, space=bass.MemorySpace.PSUM) as ps,
        ):
            # constants ride the gpsimd (SWDGE) queue: it is otherwise idle at
            # t=0, so the bias matmul + epilogue unblock within a few us while
            # the two HWDGE queues (sync: W stream, scalar: x stream) fill.
            # constants go FIRST on the two fast HWDGE queues: the epilogue of
            # the very first psum group needs them, and the SWDGE queue crawls
            # on gather patterns (measured ~8us for a 2048-packet gather).
            sC_sb = cst.tile([128, NT // 128, 2], F32)
            nc.sync.dma_start(sC_sb[:], sC[:])
            brep_sb = cst.tile([128, 2, D_HID], F32)
            nc.sync.dma_start(brep_sb[:, 0, :], brep[:, 0, :])
            nc.scalar.dma_start(brep_sb[:, 1, :], brep[:, 1, :])

            # sync + scalar are pure DMA-issue queues (no compute on either, so
            # a dma_start blocked on a tile-slot semaphore never stalls math).
            # Both W and x are split across the two HWDGE queues to halve
            # arrival latency; the q==0 x-slice is emitted ahead of the W
            # block so a new nb-block never starts x-starved.
            def load_x(q):
                x_t = []
                for k in range(KCH):
                    t = xp.tile([KP, TQ], X_DT, tag=f"x{k}")
                    eng = nc.sync if k % 2 == 0 else nc.scalar
                    eng.dma_start(
                        t[:], xT[k * KP:(k + 1) * KP, q * TQ:(q + 1) * TQ]
                    )
                    x_t.append(t)
                return x_t

            def load_w(nb, k, e, wd, nb_sl):
                t = wp.tile([KP, NB], W_DT, tag=f"w{e}_{k}")
                eng = nc.sync if (k + e) % 2 == 0 else nc.scalar
                eng.dma_start(t[:], wd[k * KP:(k + 1) * KP, nb_sl])
                return t

            for nb in range(NBLK):
                nb_sl = bass.ts(nb, NB)
                w_t = {}
                if nb == 0:
                    # cold start: interleave x-slice-0 and W chunks k-major so
                    # the PE can begin the k-loop as soon as chunk 0 lands.
                    x_first = []
                    for k in range(KCH):
                        t = xp.tile([KP, TQ], X_DT, tag=f"x{k}")
                        eng = nc.sync if k % 2 == 0 else nc.scalar
                        eng.dma_start(t[:], xT[k * KP:(k + 1) * KP, 0:TQ])
                        x_first.append(t)
                        for e, wd in enumerate((wa, wb)):
                            w_t[e, k] = load_w(nb, k, e, wd, nb_sl)
                else:
                    x_first = load_x(0)
                    for k in range(KCH):
                        for e, wd in enumerate((wa, wb)):
                            w_t[e, k] = load_w(nb, k, e, wd, nb_sl)
                for q in range(NQ):
                    x_t = x_first if q == 0 else load_x(q)
                    for mi in range(MPQ):
                        mg = q * MPQ + mi
                        pa = ps.tile([128, NB], F32, tag="pa")
                        pb = ps.tile([128, NB], F32, tag="pb")
                        for k in range(KCH):
                            xk = x_t[k][:, bass.ts(mi, 128)]
                            nc.tensor.matmul(
                                pa[:], xk, w_t[0, k][:],
                                start=(k == 0), stop=(k == KCH - 1),
                            )
                            nc.tensor.matmul(
                                pb[:], xk, w_t[1, k][:],
                                start=(k == 0), stop=(k == KCH - 1),
                            )
                        s0 = sC_sb[:, mg, 0:1]
                        s1 = sC_sb[:, mg, 1:2]
                        # epilogue on DVE: out = s0*(pa+bA) + s1*(pb+bB)
                        # (each op reads at most one PSUM input)
                        u = ep.tile([128, NB], F32, tag="u")
                        nc.vector.tensor_add(u[:], pa[:], brep_sb[:, 0, nb_sl])
                        t1 = ep.tile([128, NB], F32, tag="t1")
                        nc.vector.tensor_scalar_mul(t1[:], u[:], s0)
                        v = ep.tile([128, NB], F32, tag="v")
                        nc.vector.tensor_add(v[:], pb[:], brep_sb[:, 1, nb_sl])
                        o = ep.tile([128, NB], F32, tag="o")
                        nc.vector.scalar_tensor_tensor(
                            o[:], v[:], s1, t1[:], op0=MULT, op1=ADD
                        )
                        nc.gpsimd.dma_start(out[bass.ts(mg, 128), nb_sl], o[:])

    nc.compile()
    return nc


def _host_gating(x, W_gate, b_gate):
    logits = x @ W_gate + b_gate                       # [N, 8] fp32
    m = logits.max(axis=1, keepdims=True)
    e = np.exp(logits - m)
    gates = e / e.sum(axis=1, keepdims=True)
    idx0 = np.argsort(-gates[0], kind="stable")[:2]    # token-0 top-2 experts
    scores = -np.sort(-gates, axis=1)[:, :2]           # per-token top-2 values
    return idx0, np.ascontiguousarray(scores)


def kernel(x, W_experts, b_experts, W_gate, b_gate):
    global LAST_RESULT
    x = np.ascontiguousarray(np.asarray(x, dtype=np.float32))
    W_experts = np.asarray(W_experts, dtype=np.float32)
    b_experts = np.asarray(b_experts, dtype=np.float32)
    W_gate = np.asarray(W_gate, dtype=np.float32)
    b_gate = np.asarray(b_gate, dtype=np.float32)

    idx0, scores = _host_gating(x, W_gate, b_gate)
    w_np_dt = mybir.dt.np(W_DT)
    x_np_dt = mybir.dt.np(X_DT)
    wa = np.ascontiguousarray(W_experts[idx0[0]]).astype(w_np_dt)  # [D_IN, D_HID]
    wb = np.ascontiguousarray(W_experts[idx0[1]]).astype(w_np_dt)
    brep = np.ascontiguousarray(
        np.broadcast_to(b_experts[idx0][None], (128, 2, D_HID))
    ).astype(np.float32)

    xT_full = np.ascontiguousarray(x.astype(x_np_dt).T)            # [D_IN, N]

    nc = _build()
    in_maps = []
    for c in range(N_CORES):
        sl = slice(c * NT, (c + 1) * NT)
        in_maps.append(
            {
                "xT": np.ascontiguousarray(xT_full[:, sl]),
                "wa": wa,
                "wb": wb,
                "brep": brep,
                "sC": np.ascontiguousarray(
                    scores[sl].reshape(NT // 128, 128, 2).transpose(1, 0, 2)
                ),
            }
        )

    res = run_bass_kernel_spmd(nc, in_maps, list(range(N_CORES)))
    LAST_RESULT = res
    return np.concatenate([r["out"] for r in res.results], axis=0)

